# revision 27
# baseline (speedup 1.0000x reference)
"""Trainium2 Bass kernel for AtomicNumberPooling (segment-sum pooling).

Math: output[b, (z-1)*F + f] = sum_{n: batch[n]==b, z[n]==z} out[n, f],
i.e. a segment sum over combined id seg = batch*100 + (z-1), reshaped to
[B, 100*F].

Strategy (v2)
-------------
`batch` is sorted, so sharding the B=1000 graphs contiguously over 8 cores
gives each core a contiguous row range of `out` and a fully disjoint slice
of the output - no collective needed.

Host-side packing (index bookkeeping + dtype casts only):
  * rows are grouped per graph and zero-padded to GPAD=256 rows/graph
    (2 tiles of 128; the real max for this input size is 252; overflow
    rows fall back to a host-side correction);
  * x is shipped as fp16 (256B/row; pooled-output rel err ~4.5e-4 vs the
    2e-2 gate) laid out partition-major [128, NT*F];
  * the z one-hot is built ON HOST as fp8_e4m3 [128, NT*100] (values 0/1
    are exact in e4m3; 100B/row) - no DVE iota/compare and no Pool casts
    on device; padding rows get all-zero one-hot rows so they contribute
    nothing (this also makes empty graphs come out exactly zero).

Device program (per core, identical SPMD):
  * PE: one matmul per 128-row tile - x tile [128,128] fp16 stationary,
    one-hot [128,100] fp8 moving - accumulating into psum[f, zcol] for
    that tile's graph. Mixed fp16 x fp8 operands are exact here since the
    one-hot is 0/1. Graphs map to 400B psum column slots, 5 per 2KB bank
    (48B hole per bank keeps matmuls inside one bank), 4 banks per psum
    tensor, 2 tensors rotating -> blocks of 20 graphs in flight.
  * DVE: per finished block, one strided copy psum [128,4x500] f32 ->
    osb fp16 (cast), skipping the bank holes, packing graphs densely.
  * Stores: Pool (SWDGE) ships osb [128, 2000-col] fp16 slices to the
    output DRAM tensor o [128, GB*100] (partition-major [f, g*100+z];
    the host transposes when unsharding).
  * Loads are split across the three DMA-capable queues (SP, ACT, Pool)
    to balance queue occupancy: SP carries most x chunks, ACT carries the
    one-hot + the tail x chunks, Pool carries early x chunks then stores.
  * raw bass Block with explicit single-semaphore waits (the walrus build
    in this container rejects multi-sem-wait instructions); one semaphore
    per load chunk / store (DMA completions may land out of order).
"""

from contextlib import ExitStack

import ml_dtypes
import numpy as np

import concourse.bass as bass
import concourse.mybir as mybir
from concourse.bass_utils import run_bass_kernel_spmd

NCORES = 8
B = 1000
MAX_Z = 100
F = 128
TP = 128                 # rows per tile (SBUF partition dim)
GB = B // NCORES         # graphs per core
GPAD = 256               # padded rows per graph (real max for this input: 252)
SPG = GPAD // TP         # row tiles per graph (= 2)
NT = GB * SPG            # row tiles per core (= 250)
RPC = GB * GPAD          # padded rows per core (= 32000)
E4M3 = ml_dtypes.float8_e4m3

# psum layout: 5 graph slots of 100 f32 per 2KB bank (48B hole), 2 banks
# per psum tensor (1024 f32), 4 tensors rotating -> 10-graph blocks, with
# small tail blocks to shorten the end-of-pipeline copy+store chain.
SLOTS_PER_BANK = 5
BANKS_PER_PS = 2
NPS = 4                                     # psum tensors (4 x 2 banks)
BLK = SLOTS_PER_BANK * BANKS_PER_PS         # 10 graphs per full block

# small first block -> DVE copy chain starts ~0.7us earlier; small tail
# blocks -> short end-of-pipeline copy+store chain
BLOCK_SIZES = [5] + [10] * 11 + [5, 5]      # sums to GB = 125
def _ps_idx(k):
    return k % NPS


def _blocks():
    out = []
    g = 0
    for w in BLOCK_SIZES:
        out.append((g, w))
        g += w
    assert g == GB
    return out


def _psum_col(g_in_block):
    return 512 * (g_in_block // SLOTS_PER_BANK) + 100 * (g_in_block % SLOTS_PER_BANK)


# engine assignment for the tail copies/stores: DVE would otherwise be a
# serialized chain to the very end; ACT/SP/Pool are idle once loads finish
# walrus: GPSIMD cannot access PSUM, and the first ACT activation pays a
# 1383ns table load -> all psum evacuation runs on DVE; late stores ride
# the queues that drain first (SP/ACT)
COPY_ENG = ["dve"] * 12 + ["act", "act"]
STORE_ENG = ["pool"] * 12 + ["sp", "act"]

# Load schedule: spans of 20 tiles aligned with psum blocks. Pool's x
# chunks are all emitted before its stores, so every Pool tile lands in
# the first ~6.5us; SP and ACT pace the tail.
SPAN = 20


def _load_schedule():
    """Returns (sp_x, act_items, pool_x): chunk lists in program order.

    ACT streams ALL one-hot chunks first (oh(s) lands ~1.2+0.77s us,
    always ahead of PE's ~1.18us/span consumption), then picks up the x
    chunks for the last spans, which are exactly due by then. Pool
    preloads early/mid-span x before its stores; SP paces the middle."""
    sp_x = [(0, 4), (4, 8)]           # span 0: SP tiles 0..12
    act_items = [("oh", 0, 4), ("oh", 4, 16)]
    pool_x = [(12, 4), (16, 4)]       # span 0: Pool tiles 12..20 (fast start)
    for s in range(1, 12):
        act_items.append(("oh", s * SPAN, SPAN))
    act_items.append(("oh", 240, 10))
    # spans 1..9: SP 12 + Pool 8 (Pool x all precedes its stores)
    for s in range(1, 10):
        t0 = s * SPAN
        sp_x.append((t0, 12))
        pool_x.append((t0 + 12, 8))
    # spans 10,11: SP 12 + ACT 8; span 12 tail: ACT 10 (due late);
    # SP's last chunk is split so the final tiles land with less backlog
    sp_x.append((200, 12))
    act_items.append(("x", 212, 8))
    sp_x.append((220, 6))
    act_items.append(("x", 226, 6))
    act_items.append(("x", 232, 8))
    act_items.append(("x", 240, 10))
    return sp_x, act_items, pool_x


def _build(start_clear=True):
    blocks = _blocks()
    sp_x, act_items, pool_x = _load_schedule()
    act_x = [(a, w) for kind, a, w in act_items if kind == "x"]
    oh_chunks = [(a, w) for kind, a, w in act_items if kind == "oh"]
    nblk = len(blocks)
    # tile -> (block, graph-in-block, start, stop) and block mm thresholds
    tile_info = []
    for t in range(NT):
        g, s = divmod(t, SPG)
        k = next(i for i, (g0, gw) in enumerate(blocks) if g0 <= g < g0 + gw)
        tile_info.append((k, g - blocks[k][0], s == 0, s == SPG - 1))
    blk_mm_done = [(blocks[k][0] + blocks[k][1]) * SPG for k in range(nblk)]

    nc = bass.Bass()
    x = nc.dram_tensor("x", [TP, NT * F], mybir.dt.float16, kind="ExternalInput")
    oh = nc.dram_tensor("oh", [TP, NT * MAX_Z], mybir.dt.float8e4,
                        kind="ExternalInput")
    o = nc.dram_tensor("o", [TP, GB * MAX_Z], mybir.dt.float16,
                       kind="ExternalOutput")

    with ExitStack() as ctx:
        xb = ctx.enter_context(
            nc.sbuf_tensor("xb", [TP, NT * F], mybir.dt.float16))
        ohb = ctx.enter_context(
            nc.sbuf_tensor("ohb", [TP, NT * MAX_Z], mybir.dt.float8e4))
        osb = ctx.enter_context(
            nc.sbuf_tensor("osb", [TP, GB * MAX_Z], mybir.dt.float16))
        scr = ctx.enter_context(
            nc.sbuf_tensor("scr", [TP, 8], mybir.dt.float16))
        ps = [
            ctx.enter_context(
                nc.psum_tensor(f"ps{i}", [TP, 512 * BANKS_PER_PS],
                               mybir.dt.float32))
            for i in range(NPS)
        ]

        s_x = [ctx.enter_context(nc.semaphore(f"s_x{i}"))
               for i in range(len(sp_x) + len(act_x) + len(pool_x))]
        s_oh = [ctx.enter_context(nc.semaphore(f"s_oh{i}"))
                for i in range(len(oh_chunks))]
        s_mm = ctx.enter_context(nc.semaphore("s_mm"))   # +1 per tile matmul
        s_cpb = [ctx.enter_context(nc.semaphore(f"s_cpb{i}"))
                 for i in range(nblk)]                   # +1 per block copy
        s_st = {q: ctx.enter_context(nc.semaphore(f"s_st_{q}"))
                for q in ("pool", "sp", "act")}
        my_sems = [*s_x, *s_oh, s_mm, *s_cpb, *s_st.values()]

        xinfo = []
        sem_i = 0
        for q, lst in (("sp", sp_x), ("act", act_x), ("pool", pool_x)):
            for a, w in lst:
                xinfo.append((a, w, sem_i, q))
                sem_i += 1
        tile_xsem = [None] * NT
        for a, w, si, _q in xinfo:
            for t in range(a, a + w):
                tile_xsem[t] = si
        tile_ohsem = [None] * NT
        for j, (a, w) in enumerate(oh_chunks):
            for t in range(a, a + w):
                tile_ohsem[t] = j

        def emit_copy(eng, k):
            g0, gw = blocks[k]
            eng.wait_ge(s_mm, blk_mm_done[k])
            if gw % SLOTS_PER_BANK == 0:
                nbank = gw // SLOTS_PER_BANK
                src = ps[_ps_idx(k)][:, 0:512 * nbank].rearrange(
                    "p (b c) -> p b c", c=512)[:, :, 0:500]
                dst = osb[:, g0 * MAX_Z:(g0 + gw) * MAX_Z].rearrange(
                    "p (b c) -> p b c", c=500)
            else:
                assert gw < SLOTS_PER_BANK
                src = ps[_ps_idx(k)][:, 0:gw * MAX_Z]
                dst = osb[:, g0 * MAX_Z:(g0 + gw) * MAX_Z]
            if hasattr(eng, "tensor_copy"):
                eng.tensor_copy(dst, src).then_inc(s_cpb[k], 1)
            elif hasattr(eng, "tensor_scalar_mul"):
                eng.tensor_scalar_mul(dst, src, 1.0).then_inc(s_cpb[k], 1)
            else:
                eng.copy(dst, src).then_inc(s_cpb[k], 1)

        def emit_store(eng, k):
            g0, gw = blocks[k]
            eng.wait_ge(s_cpb[k], 1)
            eng.dma_start(
                o[:, g0 * MAX_Z:(g0 + gw) * MAX_Z],
                osb[:, g0 * MAX_Z:(g0 + gw) * MAX_Z],
            ).then_inc(s_st[STORE_ENG[k]], 16)

        if start_clear:
            nc.gpsimd.dma_reset()
            for s in my_sems:
                nc.gpsimd.sem_clear(s)
            nc._nrt_pseudo_barrier()

        with nc.Block() as block:

            @block.sync
            def _(sync):
                for a, w, si, q in xinfo:
                    if q != "sp":
                        continue
                    sync.dma_start(
                        xb[:, a * F:(a + w) * F],
                        x[:, a * F:(a + w) * F],
                    ).then_inc(s_x[si], 16)
                for k in range(nblk):
                    if STORE_ENG[k] == "sp":
                        emit_store(sync, k)
                n_sp = sum(1 for e in STORE_ENG if e == "sp")
                if n_sp:
                    sync.wait_ge(s_st["sp"], 16 * n_sp)

            @block.scalar
            def _(scalar):
                oh_j = 0
                act_si = iter([si for a, w, si, q in xinfo if q == "act"])
                for kind, a, w in act_items:
                    if kind == "oh":
                        scalar.dma_start(
                            ohb[:, a * MAX_Z:(a + w) * MAX_Z],
                            oh[:, a * MAX_Z:(a + w) * MAX_Z],
                        ).then_inc(s_oh[oh_j], 16)
                        oh_j += 1
                    else:
                        scalar.dma_start(
                            xb[:, a * F:(a + w) * F],
                            x[:, a * F:(a + w) * F],
                        ).then_inc(s_x[next(act_si)], 16)
                # prepay the one-time ACT activation-table load while idle
                # (the first InstActivation costs +1383ns; do it off the
                # critical path before the tail psum copies)
                scalar.wait_ge(s_x[0], 16)
                scalar.copy(scr[:], xb[:, 0:8])
                # tail copies + stores interleaved in block order
                for k in range(nblk):
                    if COPY_ENG[k] == "act":
                        emit_copy(scalar, k)
                    if STORE_ENG[k] == "act":
                        emit_store(scalar, k)
                n_act = sum(1 for e in STORE_ENG if e == "act")
                if n_act:
                    scalar.wait_ge(s_st["act"], 16 * n_act)

            @block.gpsimd
            def _(gpsimd):
                for a, w, si, q in xinfo:
                    if q != "pool":
                        continue
                    gpsimd.dma_start(
                        xb[:, a * F:(a + w) * F],
                        x[:, a * F:(a + w) * F],
                    ).then_inc(s_x[si], 16)
                for k in range(nblk):
                    if COPY_ENG[k] == "pool":
                        emit_copy(gpsimd, k)
                    if STORE_ENG[k] == "pool":
                        emit_store(gpsimd, k)
                n_pool = sum(1 for e in STORE_ENG if e == "pool")
                if n_pool:
                    gpsimd.wait_ge(s_st["pool"], 16 * n_pool)

            @block.tensor
            def _(tensor):
                seen_x = set()
                seen_oh = set()
                for t in range(NT):
                    k, gq, st0, st1 = tile_info[t]
                    xs = tile_xsem[t]
                    os_ = tile_ohsem[t]
                    if xs not in seen_x:
                        tensor.wait_ge(s_x[xs], 16)
                        seen_x.add(xs)
                    if os_ not in seen_oh:
                        tensor.wait_ge(s_oh[os_], 16)
                        seen_oh.add(os_)
                    if st0 and gq == 0 and k >= NPS:
                        tensor.wait_ge(s_cpb[k - NPS], 1)      # psum free
                    col = _psum_col(gq)
                    tensor.matmul(
                        ps[_ps_idx(k)][:, col:col + MAX_Z],
                        xb[:, t * F:(t + 1) * F],
                        ohb[:, t * MAX_Z:(t + 1) * MAX_Z],
                        start=st0, stop=st1,
                    ).then_inc(s_mm, 1)

            @block.vector
            def _(vector):
                for k in range(nblk):
                    if COPY_ENG[k] == "dve":
                        emit_copy(vector, k)

        # Block exit emitted an all-engine barrier: everything is quiesced.
        # Clear sems for the next execution, split across engines so the
        # trailing cleanup is ~4x shorter.
        engs = [nc.gpsimd, nc.sync, nc.scalar, nc.vector]
        for i, s in enumerate(my_sems):
            engs[i % 4].sem_clear(s)

    return nc


_NC = None


def _get_nc():
    global _NC
    if _NC is None:
        _NC = _build()
    return _NC


def _pack_inputs(x, z, b):
    """Build per-core input maps; returns (in_maps, host_fix).

    host_fix is a [B*MAX_Z, F] float32 correction for rows that could not
    be placed on the device (graph overflow beyond GPAD) - all zeros for
    sane inputs; kept for robustness.
    """
    in_maps = []
    host_fix = None
    zcol = z.astype(np.int64) - 1
    x16 = x.astype(np.float16)
    for c in range(NCORES):
        g_lo, g_hi = c * GB, (c + 1) * GB
        r0 = np.searchsorted(b, g_lo, side="left")
        r1 = np.searchsorted(b, g_hi, side="left")
        bb = (b[r0:r1] - g_lo).astype(np.int64)
        zz = zcol[r0:r1]
        hh = x16[r0:r1]

        cnt = np.bincount(bb, minlength=GB)
        offs = np.zeros(GB + 1, np.int64)
        offs[1:] = np.cumsum(cnt)
        rank = np.arange(len(bb)) - offs[bb]

        zok = (zz >= 0) & (zz < MAX_Z)
        ok = (rank < GPAD) & zok
        if not (rank < GPAD).all():
            # overflow rows: accumulate on host (never hit for this dataset)
            if host_fix is None:
                host_fix = np.zeros((B * MAX_Z, F), np.float32)
            sel = (~(rank < GPAD)) & zok
            seg = (b[r0:r1][sel].astype(np.int64) * MAX_Z + zz[sel])
            np.add.at(host_fix, seg, x[r0:r1][sel])
        bb, zz, hh, rank = bb[ok], zz[ok], hh[ok], rank[ok]

        dest = bb * GPAD + rank
        xp = np.zeros((RPC, F), np.float16)
        xp[dest] = hh
        ohp = np.zeros((RPC, MAX_Z), E4M3)
        ohp[dest, zz] = E4M3(1.0)
        # partition-major: row r -> [r % 128, (r // 128)*W : ...]
        xm = np.ascontiguousarray(
            xp.reshape(NT, TP, F).transpose(1, 0, 2).reshape(TP, NT * F))
        ohm = np.ascontiguousarray(
            ohp.reshape(NT, TP, MAX_Z).transpose(1, 0, 2)
            .reshape(TP, NT * MAX_Z))
        in_maps.append({"x": xm, "oh": ohm})
    return in_maps, host_fix


def kernel(out, z, batch):
    x = np.asarray(out, dtype=np.float32)
    z = np.asarray(z)
    b = np.asarray(batch)

    if np.any(b[1:] < b[:-1]):                # robustness: ensure sorted
        order = np.argsort(b, kind="stable")
        x, z, b = x[order], z[order], b[order]
    valid = (b >= 0) & (b < B)                # out-of-range graphs: dropped
    if not valid.all():
        x, z, b = x[valid], z[valid], b[valid]

    in_maps, host_fix = _pack_inputs(x, z, b)
    res = run_bass_kernel_spmd(_get_nc(), in_maps, list(range(NCORES)))
    # device output is partition-major [F, GB*MAX_Z]; transpose to
    # [GB*MAX_Z, F] per core while gathering
    blocks = [
        np.ascontiguousarray(res.results[c]["o"].T).astype(np.float32)
        for c in range(NCORES)
    ]
    pooled = np.concatenate(blocks, axis=0)
    if host_fix is not None:
        pooled = pooled + host_fix
    return pooled.reshape(B, MAX_Z * F)


# revision 30
# speedup vs baseline: 1.0325x; 1.0325x over previous
"""Trainium2 Bass kernel for AtomicNumberPooling (segment-sum pooling).

Math: output[b, (z-1)*F + f] = sum_{n: batch[n]==b, z[n]==z} out[n, f],
i.e. a segment sum over combined id seg = batch*100 + (z-1), reshaped to
[B, 100*F].

Strategy (v2)
-------------
`batch` is sorted, so sharding the B=1000 graphs contiguously over 8 cores
gives each core a contiguous row range of `out` and a fully disjoint slice
of the output - no collective needed.

Host-side packing (index bookkeeping + dtype casts only):
  * rows are grouped per graph and zero-padded to GPAD=256 rows/graph
    (2 tiles of 128; the real max for this input size is 252; overflow
    rows fall back to a host-side correction);
  * x is shipped as fp16 (256B/row; pooled-output rel err ~4.5e-4 vs the
    2e-2 gate) laid out partition-major [128, NT*F];
  * the z one-hot is built ON HOST as fp8_e4m3 [128, NT*100] (values 0/1
    are exact in e4m3; 100B/row) - no DVE iota/compare and no Pool casts
    on device; padding rows get all-zero one-hot rows so they contribute
    nothing (this also makes empty graphs come out exactly zero).

Device program (per core, identical SPMD):
  * PE: one matmul per 128-row tile - x tile [128,128] fp16 stationary,
    one-hot [128,100] fp8 moving - accumulating into psum[f, zcol] for
    that tile's graph. Mixed fp16 x fp8 operands are exact here since the
    one-hot is 0/1. Graphs map to 400B psum column slots, 5 per 2KB bank
    (48B hole per bank keeps matmuls inside one bank), 4 banks per psum
    tensor, 2 tensors rotating -> blocks of 20 graphs in flight.
  * DVE: per finished block, one strided copy psum [128,4x500] f32 ->
    osb fp16 (cast), skipping the bank holes, packing graphs densely.
  * Stores: Pool (SWDGE) ships osb [128, 2000-col] fp16 slices to the
    output DRAM tensor o [128, GB*100] (partition-major [f, g*100+z];
    the host transposes when unsharding).
  * Loads are split across the three DMA-capable queues (SP, ACT, Pool)
    to balance queue occupancy: SP carries most x chunks, ACT carries the
    one-hot + the tail x chunks, Pool carries early x chunks then stores.
  * raw bass Block with explicit single-semaphore waits (the walrus build
    in this container rejects multi-sem-wait instructions); one semaphore
    per load chunk / store (DMA completions may land out of order).
"""

from contextlib import ExitStack

import ml_dtypes
import numpy as np

import concourse.bass as bass
import concourse.mybir as mybir
from concourse.bass_utils import run_bass_kernel_spmd

NCORES = 8
B = 1000
MAX_Z = 100
F = 128
TP = 128                 # rows per tile (SBUF partition dim)
GB = B // NCORES         # graphs per core
GPAD = 256               # padded rows per graph (real max for this input: 252)
SPG = GPAD // TP         # row tiles per graph (= 2)
NT = GB * SPG            # row tiles per core (= 250)
RPC = GB * GPAD          # padded rows per core (= 32000)
E4M3 = ml_dtypes.float8_e4m3

# psum layout: 5 graph slots of 100 f32 per 2KB bank (48B hole), 2 banks
# per psum tensor (1024 f32), 4 tensors rotating -> 10-graph blocks, with
# small tail blocks to shorten the end-of-pipeline copy+store chain.
SLOTS_PER_BANK = 5
BANKS_PER_PS = 2
NPS = 4                                     # psum tensors (4 x 2 banks)
BLK = SLOTS_PER_BANK * BANKS_PER_PS         # 10 graphs per full block

# small first block -> DVE copy chain starts ~0.7us earlier; small tail
# blocks -> short end-of-pipeline copy+store chain
BLOCK_SIZES = [5] + [10] * 11 + [5, 5]      # sums to GB = 125
def _ps_idx(k):
    return k % NPS


def _blocks():
    out = []
    g = 0
    for w in BLOCK_SIZES:
        out.append((g, w))
        g += w
    assert g == GB
    return out


def _psum_col(g_in_block):
    return 512 * (g_in_block // SLOTS_PER_BANK) + 100 * (g_in_block % SLOTS_PER_BANK)


# engine assignment for the tail copies/stores: DVE would otherwise be a
# serialized chain to the very end; ACT/SP/Pool are idle once loads finish
# walrus: GPSIMD cannot access PSUM, and the first ACT activation pays a
# 1383ns table load -> all psum evacuation runs on DVE; late stores ride
# the queues that drain first (SP/ACT)
COPY_ENG = ["dve"] * 10 + ["act"] + ["dve"] + ["act", "act"]
STORE_ENG = ["pool"] * 12 + ["sp", "act"]

# Load schedule: spans of 20 tiles aligned with psum blocks. Pool's x
# chunks are all emitted before its stores, so every Pool tile lands in
# the first ~6.5us; SP and ACT pace the tail.
SPAN = 20


def _load_schedule():
    """Returns (sp_x, act_items, pool_x): chunk lists in program order.

    ACT streams ALL one-hot chunks first (oh(s) lands ~1.2+0.77s us,
    always ahead of PE's ~1.18us/span consumption), then picks up the x
    chunks for the last spans, which are exactly due by then. Pool
    preloads early/mid-span x before its stores; SP paces the middle."""
    sp_x = [(0, 4), (4, 8)]           # span 0: SP tiles 0..12
    act_items = [("oh", 0, 4), ("oh", 4, 16)]
    pool_x = [(12, 4), (16, 4)]       # span 0: Pool tiles 12..20 (fast start)
    for s in range(1, 12):
        act_items.append(("oh", s * SPAN, SPAN))
    act_items.append(("oh", 240, 10))
    # spans 1..9: SP 12 + Pool 8 (Pool x all precedes its stores)
    for s in range(1, 10):
        t0 = s * SPAN
        sp_x.append((t0, 12))
        pool_x.append((t0 + 12, 8))
    # spans 10,11: SP 12 + ACT 8; span 12 tail: ACT 10 (due late);
    # SP's last chunk is split so the final tiles land with less backlog
    sp_x.append((200, 12))
    act_items.append(("x", 212, 8))
    sp_x.append((220, 6))
    act_items.append(("x", 226, 6))
    act_items.append(("x", 232, 8))
    act_items.append(("x", 240, 10))
    return sp_x, act_items, pool_x


def _build(start_clear=True):
    blocks = _blocks()
    sp_x, act_items, pool_x = _load_schedule()
    act_x = [(a, w) for kind, a, w in act_items if kind == "x"]
    oh_chunks = [(a, w) for kind, a, w in act_items if kind == "oh"]
    nblk = len(blocks)
    # tile -> (block, graph-in-block, start, stop) and block mm thresholds
    tile_info = []
    for t in range(NT):
        g, s = divmod(t, SPG)
        k = next(i for i, (g0, gw) in enumerate(blocks) if g0 <= g < g0 + gw)
        tile_info.append((k, g - blocks[k][0], s == 0, s == SPG - 1))
    blk_mm_done = [(blocks[k][0] + blocks[k][1]) * SPG for k in range(nblk)]

    nc = bass.Bass()
    x = nc.dram_tensor("x", [TP, NT * F], mybir.dt.float16, kind="ExternalInput")
    oh = nc.dram_tensor("oh", [TP, NT * MAX_Z], mybir.dt.float8e4,
                        kind="ExternalInput")
    o = nc.dram_tensor("o", [TP, GB * MAX_Z], mybir.dt.float16,
                       kind="ExternalOutput")

    with ExitStack() as ctx:
        xb = ctx.enter_context(
            nc.sbuf_tensor("xb", [TP, NT * F], mybir.dt.float16))
        ohb = ctx.enter_context(
            nc.sbuf_tensor("ohb", [TP, NT * MAX_Z], mybir.dt.float8e4))
        osb = ctx.enter_context(
            nc.sbuf_tensor("osb", [TP, GB * MAX_Z], mybir.dt.float16))
        scr = ctx.enter_context(
            nc.sbuf_tensor("scr", [TP, 8], mybir.dt.float16))
        ps = [
            ctx.enter_context(
                nc.psum_tensor(f"ps{i}", [TP, 512 * BANKS_PER_PS],
                               mybir.dt.float32))
            for i in range(NPS)
        ]

        s_x = [ctx.enter_context(nc.semaphore(f"s_x{i}"))
               for i in range(len(sp_x) + len(act_x) + len(pool_x))]
        s_oh = [ctx.enter_context(nc.semaphore(f"s_oh{i}"))
                for i in range(len(oh_chunks))]
        s_mm = ctx.enter_context(nc.semaphore("s_mm"))   # +1 per tile matmul
        s_cpb = [ctx.enter_context(nc.semaphore(f"s_cpb{i}"))
                 for i in range(nblk)]                   # +1 per block copy
        s_st = {q: ctx.enter_context(nc.semaphore(f"s_st_{q}"))
                for q in ("pool", "sp", "act")}
        my_sems = [*s_x, *s_oh, s_mm, *s_cpb, *s_st.values()]

        xinfo = []
        sem_i = 0
        for q, lst in (("sp", sp_x), ("act", act_x), ("pool", pool_x)):
            for a, w in lst:
                xinfo.append((a, w, sem_i, q))
                sem_i += 1
        tile_xsem = [None] * NT
        for a, w, si, _q in xinfo:
            for t in range(a, a + w):
                tile_xsem[t] = si
        tile_ohsem = [None] * NT
        for j, (a, w) in enumerate(oh_chunks):
            for t in range(a, a + w):
                tile_ohsem[t] = j

        def emit_copy(eng, k):
            g0, gw = blocks[k]
            eng.wait_ge(s_mm, blk_mm_done[k])
            if gw % SLOTS_PER_BANK == 0:
                nbank = gw // SLOTS_PER_BANK
                src = ps[_ps_idx(k)][:, 0:512 * nbank].rearrange(
                    "p (b c) -> p b c", c=512)[:, :, 0:500]
                dst = osb[:, g0 * MAX_Z:(g0 + gw) * MAX_Z].rearrange(
                    "p (b c) -> p b c", c=500)
            else:
                assert gw < SLOTS_PER_BANK
                src = ps[_ps_idx(k)][:, 0:gw * MAX_Z]
                dst = osb[:, g0 * MAX_Z:(g0 + gw) * MAX_Z]
            if hasattr(eng, "tensor_copy"):
                eng.tensor_copy(dst, src).then_inc(s_cpb[k], 1)
            elif hasattr(eng, "tensor_scalar_mul"):
                eng.tensor_scalar_mul(dst, src, 1.0).then_inc(s_cpb[k], 1)
            else:
                eng.copy(dst, src).then_inc(s_cpb[k], 1)

        def emit_store(eng, k):
            g0, gw = blocks[k]
            eng.wait_ge(s_cpb[k], 1)
            eng.dma_start(
                o[:, g0 * MAX_Z:(g0 + gw) * MAX_Z],
                osb[:, g0 * MAX_Z:(g0 + gw) * MAX_Z],
            ).then_inc(s_st[STORE_ENG[k]], 16)

        if start_clear:
            nc.gpsimd.dma_reset()
            for s in my_sems:
                nc.gpsimd.sem_clear(s)
            nc._nrt_pseudo_barrier()

        with nc.Block() as block:

            @block.sync
            def _(sync):
                for a, w, si, q in xinfo:
                    if q != "sp":
                        continue
                    sync.dma_start(
                        xb[:, a * F:(a + w) * F],
                        x[:, a * F:(a + w) * F],
                    ).then_inc(s_x[si], 16)
                for k in range(nblk):
                    if STORE_ENG[k] == "sp":
                        emit_store(sync, k)
                n_sp = sum(1 for e in STORE_ENG if e == "sp")
                if n_sp:
                    sync.wait_ge(s_st["sp"], 16 * n_sp)

            @block.scalar
            def _(scalar):
                oh_j = 0
                act_si = iter([si for a, w, si, q in xinfo if q == "act"])
                for kind, a, w in act_items:
                    if kind == "oh":
                        scalar.dma_start(
                            ohb[:, a * MAX_Z:(a + w) * MAX_Z],
                            oh[:, a * MAX_Z:(a + w) * MAX_Z],
                        ).then_inc(s_oh[oh_j], 16)
                        oh_j += 1
                    else:
                        scalar.dma_start(
                            xb[:, a * F:(a + w) * F],
                            x[:, a * F:(a + w) * F],
                        ).then_inc(s_x[next(act_si)], 16)
                # prepay the one-time ACT activation-table load while idle
                # (the first InstActivation costs +1383ns; do it off the
                # critical path before the tail psum copies)
                scalar.wait_ge(s_x[0], 16)
                scalar.copy(scr[:], xb[:, 0:8])
                # tail copies + stores interleaved in block order
                for k in range(nblk):
                    if COPY_ENG[k] == "act":
                        emit_copy(scalar, k)
                    if STORE_ENG[k] == "act":
                        emit_store(scalar, k)
                n_act = sum(1 for e in STORE_ENG if e == "act")
                if n_act:
                    scalar.wait_ge(s_st["act"], 16 * n_act)

            @block.gpsimd
            def _(gpsimd):
                for a, w, si, q in xinfo:
                    if q != "pool":
                        continue
                    gpsimd.dma_start(
                        xb[:, a * F:(a + w) * F],
                        x[:, a * F:(a + w) * F],
                    ).then_inc(s_x[si], 16)
                for k in range(nblk):
                    if COPY_ENG[k] == "pool":
                        emit_copy(gpsimd, k)
                    if STORE_ENG[k] == "pool":
                        emit_store(gpsimd, k)
                n_pool = sum(1 for e in STORE_ENG if e == "pool")
                if n_pool:
                    gpsimd.wait_ge(s_st["pool"], 16 * n_pool)

            @block.tensor
            def _(tensor):
                seen_x = set()
                seen_oh = set()
                for t in range(NT):
                    k, gq, st0, st1 = tile_info[t]
                    xs = tile_xsem[t]
                    os_ = tile_ohsem[t]
                    if xs not in seen_x:
                        tensor.wait_ge(s_x[xs], 16)
                        seen_x.add(xs)
                    if os_ not in seen_oh:
                        tensor.wait_ge(s_oh[os_], 16)
                        seen_oh.add(os_)
                    if st0 and gq == 0 and k >= NPS:
                        tensor.wait_ge(s_cpb[k - NPS], 1)      # psum free
                    col = _psum_col(gq)
                    tensor.matmul(
                        ps[_ps_idx(k)][:, col:col + MAX_Z],
                        xb[:, t * F:(t + 1) * F],
                        ohb[:, t * MAX_Z:(t + 1) * MAX_Z],
                        start=st0, stop=st1,
                    ).then_inc(s_mm, 1)

            @block.vector
            def _(vector):
                for k in range(nblk):
                    if COPY_ENG[k] == "dve":
                        emit_copy(vector, k)

        # Block exit emitted an all-engine barrier: everything is quiesced.
        # Clear sems for the next execution, split across engines so the
        # trailing cleanup is ~4x shorter.
        engs = [nc.gpsimd, nc.sync, nc.scalar, nc.vector]
        for i, s in enumerate(my_sems):
            engs[i % 4].sem_clear(s)

    return nc


_NC = None


def _get_nc():
    global _NC
    if _NC is None:
        _NC = _build()
    return _NC


def _pack_inputs(x, z, b):
    """Build per-core input maps; returns (in_maps, host_fix).

    host_fix is a [B*MAX_Z, F] float32 correction for rows that could not
    be placed on the device (graph overflow beyond GPAD) - all zeros for
    sane inputs; kept for robustness.
    """
    in_maps = []
    host_fix = None
    zcol = z.astype(np.int64) - 1
    x16 = x.astype(np.float16)
    for c in range(NCORES):
        g_lo, g_hi = c * GB, (c + 1) * GB
        r0 = np.searchsorted(b, g_lo, side="left")
        r1 = np.searchsorted(b, g_hi, side="left")
        bb = (b[r0:r1] - g_lo).astype(np.int64)
        zz = zcol[r0:r1]
        hh = x16[r0:r1]

        cnt = np.bincount(bb, minlength=GB)
        offs = np.zeros(GB + 1, np.int64)
        offs[1:] = np.cumsum(cnt)
        rank = np.arange(len(bb)) - offs[bb]

        zok = (zz >= 0) & (zz < MAX_Z)
        ok = (rank < GPAD) & zok
        if not (rank < GPAD).all():
            # overflow rows: accumulate on host (never hit for this dataset)
            if host_fix is None:
                host_fix = np.zeros((B * MAX_Z, F), np.float32)
            sel = (~(rank < GPAD)) & zok
            seg = (b[r0:r1][sel].astype(np.int64) * MAX_Z + zz[sel])
            np.add.at(host_fix, seg, x[r0:r1][sel])
        bb, zz, hh, rank = bb[ok], zz[ok], hh[ok], rank[ok]

        dest = bb * GPAD + rank
        xp = np.zeros((RPC, F), np.float16)
        xp[dest] = hh
        ohp = np.zeros((RPC, MAX_Z), E4M3)
        ohp[dest, zz] = E4M3(1.0)
        # partition-major: row r -> [r % 128, (r // 128)*W : ...]
        xm = np.ascontiguousarray(
            xp.reshape(NT, TP, F).transpose(1, 0, 2).reshape(TP, NT * F))
        ohm = np.ascontiguousarray(
            ohp.reshape(NT, TP, MAX_Z).transpose(1, 0, 2)
            .reshape(TP, NT * MAX_Z))
        in_maps.append({"x": xm, "oh": ohm})
    return in_maps, host_fix


def kernel(out, z, batch):
    x = np.asarray(out, dtype=np.float32)
    z = np.asarray(z)
    b = np.asarray(batch)

    if np.any(b[1:] < b[:-1]):                # robustness: ensure sorted
        order = np.argsort(b, kind="stable")
        x, z, b = x[order], z[order], b[order]
    valid = (b >= 0) & (b < B)                # out-of-range graphs: dropped
    if not valid.all():
        x, z, b = x[valid], z[valid], b[valid]

    in_maps, host_fix = _pack_inputs(x, z, b)
    res = run_bass_kernel_spmd(_get_nc(), in_maps, list(range(NCORES)))
    # device output is partition-major [F, GB*MAX_Z]; transpose to
    # [GB*MAX_Z, F] per core while gathering
    blocks = [
        np.ascontiguousarray(res.results[c]["o"].T).astype(np.float32)
        for c in range(NCORES)
    ]
    pooled = np.concatenate(blocks, axis=0)
    if host_fix is not None:
        pooled = pooled + host_fix
    return pooled.reshape(B, MAX_Z * F)
